# revision 5
# baseline (speedup 1.0000x reference)
"""AutoRound GPTQ int4 linear on 8 TRN2 NeuronCores.

y = x @ dequant(qweight, qzeros, scales), column-parallel over out_features:
each core owns a [4096, 1376] weight shard, dequantizes it on-chip (int4
unpack + zero/scale in fp16), and runs an fp16 matmul with fp32 PSUM
accumulation. x is replicated; outputs are concatenated.

v3 dequant pipeline (the startup window where the PE would otherwise starve
waiting for weights is the main loss):
- qweight reaches the device as uint16 rows gathered on the host so that
  row k holds the u16 containing weight k's nibble (layout-only gather);
  one plain [128, 1376] DMA per k-tile.
- Dequant is split across engines, all DVE ops on 16-bit operands (2x
  mode): q = (q >> 4*(k%4)) & 15 (one fused tensor_scalar on DVE), uint16
  -> fp16 numeric cast on the scalar engine, then subtract-zero and
  multiply-scale on DVE.
- scales rows are replicated to all partitions by stride-0 DMA access
  patterns, 4 group rows per DMA; zeros rows are unpacked once, written to
  a DRAM scratch, and read back the same way (no per-tile gpsimd work).
- x is DMAd in [128, 512] fp32 slabs (2 m-blocks per slab), cast to fp16 on
  the scalar engine; the first slab pair is emitted interleaved with the
  dequant loop so no queue is head-of-line blocked.

Main loop is k-outer over 256-row m-blocks: per k, one [128, 256] fp16 xT
slice is the stationary operand of 6 matmuls (2 m-tiles x 3 out-chunks)
accumulating into 6 PSUM banks; PSUM drains split scalar/vector engine.
"""

import sys

sys.path.insert(0, "/opt/trn_rl_repo")

import numpy as np

import concourse.bacc as bacc
import concourse.mybir as mybir
import concourse.tile as tile
from concourse.bass_utils import run_bass_kernel_spmd

IN_F = 4096
OUT_F = 11008
G = 32  # quant groups (group size 128 == one k-tile)
N_CORES = 8
OUT_SHARD = OUT_F // N_CORES  # 1376
B, S = 4, 2048
M_ROWS = B * S  # 8192
M_BLK = 128
SLAB = 512  # x DMA slab width (2 m-blocks)

f32 = mybir.dt.float32
f16 = mybir.dt.float16
i32 = mybir.dt.int32
u16 = mybir.dt.uint16
Alu = mybir.AluOpType


def build_nc(m_rows=M_ROWS, out_shard=OUT_SHARD, in_f=IN_F):
    KT = in_f // 128  # k-tiles; each k-tile is exactly one quant group
    NB = m_rows // M_BLK
    assert KT == G and m_rows % SLAB == 0 and out_shard % 8 == 0

    chunks = []
    o = 0
    while o < out_shard:
        w = min(512, out_shard - o)
        chunks.append((o, w))
        o += w
    n_mt = M_BLK // 128  # m-tiles per block (2)

    nc = bacc.Bacc("TRN2", target_bir_lowering=False)
    xt_d = nc.dram_tensor("xt", (in_f, m_rows), f32, kind="ExternalInput")
    qw_d = nc.dram_tensor("qweight", (in_f, out_shard), u16, kind="ExternalInput")
    qz_d = nc.dram_tensor("qzeros", (G, out_shard // 8), i32, kind="ExternalInput")
    s_d = nc.dram_tensor("scales", (G, out_shard), f16, kind="ExternalInput")
    out_d = nc.dram_tensor("out", (m_rows, out_shard), f32, kind="ExternalOutput")
    zs_d = nc.dram_tensor("zscratch", (G, out_shard), f16, kind="Internal")

    with tile.TileContext(nc) as tc:
        with (
            tc.tile_pool(name="const", bufs=1) as cpool,
            tc.tile_pool(name="wpool", bufs=KT) as wpool,
            tc.tile_pool(name="qrep_p", bufs=3) as qrep_pool,
            tc.tile_pool(name="sb_p", bufs=3) as sb_pool,
            tc.tile_pool(name="zb_p", bufs=3) as zb_pool,
            tc.tile_pool(name="xk_p", bufs=4) as xk_pool,
            tc.tile_pool(name="xkh_p", bufs=48) as xkh_pool,
            tc.tile_pool(name="out_p", bufs=6) as out_pool,
            tc.tile_pool(name="pout", bufs=8, space="PSUM") as pout_pool,
        ):
            # --- constants ---
            iota_t = cpool.tile([128, 1], i32, tag="iota")
            nc.gpsimd.iota(iota_t[:], pattern=[[0, 1]], base=0, channel_multiplier=4)
            # per-partition nibble shift within a u16: 4*(p % 4)
            shift_ap = cpool.tile([128, 1], i32, tag="shift")
            nc.vector.tensor_scalar(shift_ap[:], iota_t[:], 12, None, Alu.bitwise_and)

            qz_sb = cpool.tile([G, out_shard // 8], i32, tag="qz_sb")
            nc.sync.dma_start(qz_sb[:], qz_d[:])
            # unpack zeros along the free dim (int-only: bitvec ops cannot cast)
            z_sbi = cpool.tile([G, out_shard], i32, tag="z_sbi")
            z_r = z_sbi[:].rearrange("g (r i) -> g r i", i=8)
            for i in range(8):
                nc.vector.tensor_scalar(
                    z_r[0:2, :, i], qz_sb[0:2, :], 4 * i, 15,
                    Alu.logical_shift_right, Alu.bitwise_and,
                )
            # Only rows 0-1 (batch 0, broadcast from SBUF) are prepared
            # here; the full unpack + DRAM staging is emitted after dequant
            # iteration 1 so the SP DMA ring isn't head-of-line blocked on
            # the zeros data dependency before qu0/xk0 can issue.
            z_sbh = cpool.tile([G, out_shard], f16, tag="z_sbh")
            nc.vector.tensor_copy(z_sbh[0:2, :], z_sbi[0:2, :])

            # --- PE warm-up: keep the tensor engine busy through the
            # dequant/DMA prologue so the first real matmuls run at full
            # clock (p-state ramps after ~3us of sustained activity) ---
            wu = cpool.tile([128, 512], f16, tag="wu")
            nc.vector.memset(wu[:], 0.0)
            wu_ps = pout_pool.tile([128, 512], f32, tag="po", name="wu_ps")
            for i in range(12):
                nc.tensor.matmul(
                    wu_ps[:], wu[:, 0:128], wu[:],
                    start=(i == 0), stop=(i == 11),
                )

            # --- dequantize weight shard into SBUF (fp16, [k, n] layout);
            # the first x slab pair loads/casts interleaved ---
            w_tiles = []
            sb4 = zb4 = None
            xkh_slabs = {}
            for t in range(KT):
                if t == 2:
                    # full zeros unpack + DRAM staging, emitted late so the
                    # first qu/x DMAs aren't queued behind its data wait
                    for i in range(8):
                        nc.vector.tensor_scalar(
                            z_r[:, :, i], qz_sb[:], 4 * i, 15,
                            Alu.logical_shift_right, Alu.bitwise_and,
                        )
                    nc.vector.tensor_copy(z_sbh[:], z_sbi[:])
                    nc.sync.dma_start(zs_d[2:, :], z_sbh[2:, :])
                if t % 2 == 0:
                    # scales / zeros: 2 group rows per replicating DMA (each
                    # row broadcast to all 128 partitions by a stride-0 dim)
                    sb4 = sb_pool.tile([128, 2 * out_shard], f16, tag="sb")
                    nc.scalar.dma_start(
                        sb4[:].rearrange("p (r n) -> p r n", r=2),
                        s_d[t : t + 2, :].unsqueeze(0).broadcast_to(
                            (128, 2, out_shard)
                        ),
                    )
                    zb4 = zb_pool.tile([128, 2 * out_shard], f16, tag="zb")
                    if t == 0:
                        # batch 0 skips the DRAM scratch round trip: row 0
                        # already sits on partition 0; row 1 via a tiny row
                        # DMA. gpsimd is idle during the prologue.
                        nc.gpsimd.partition_broadcast(
                            zb4[:, 0:out_shard], z_sbh[0:1, :]
                        )
                        zrow1 = cpool.tile([1, out_shard], f16, tag="zrow1")
                        nc.sync.dma_start(zrow1[:], z_sbh[1:2, :])
                        nc.gpsimd.partition_broadcast(
                            zb4[:, out_shard : 2 * out_shard], zrow1[:]
                        )
                    else:
                        nc.sync.dma_start(
                            zb4[:].rearrange("p (r n) -> p r n", r=2),
                            zs_d[t : t + 2, :].unsqueeze(0).broadcast_to(
                                (128, 2, out_shard)
                            ),
                        )
                # packed u16 rows pre-gathered on host: row k holds the u16
                # containing weight k's int4 at nibble 4*(k%4)
                qu = qrep_pool.tile([128, out_shard], u16, tag="qrep")
                nc.sync.dma_start(qu[:], qw_d[128 * t : 128 * (t + 1), :])
                # dequant: one fused u16 unpack on DVE (2x mode), numeric
                # uint16->fp16 cast on the scalar engine, zero/scale on DVE
                nc.vector.tensor_scalar(
                    qu[:], qu[:], shift_ap[:], 15,
                    Alu.logical_shift_right, Alu.bitwise_and,
                )
                w_t = wpool.tile([128, out_shard], f16, tag="w")
                nc.scalar.copy(w_t[:], qu[:])
                tt = t % 2
                nc.vector.tensor_tensor(
                    w_t[:], w_t[:],
                    zb4[:, tt * out_shard : (tt + 1) * out_shard],
                    Alu.subtract,
                )
                # scale-multiply: DVE for the first tiles (latency — they
                # gate the first matmuls), gpsimd afterwards (keeps DVE free;
                # measured better than all-DVE, a column split, or 2-tile
                # batched wide ops)
                mul_eng = nc.vector if t < 4 else nc.gpsimd
                mul_eng.tensor_tensor(
                    w_t[:], w_t[:],
                    sb4[:, tt * out_shard : (tt + 1) * out_shard],
                    Alu.mult,
                )
                w_tiles.append(w_t)
                # interleave the first slab pair's x loads/casts so neither
                # the SP DMA ring nor the ACT queue is head-of-line blocked
                xk = xk_pool.tile([128, SLAB], f32, tag="xk")
                nc.sync.dma_start(xk[:], xt_d[t * 128 : (t + 1) * 128, 0:SLAB])
                xkh = xkh_pool.tile([128, SLAB], f16, tag="xkh")
                nc.scalar.copy(xkh[:], xk[:])
                xkh_slabs[t] = xkh


            # --- main loop: k-outer over 128-row m-tiles, x in 512 slabs
            # (pair 0's slabs were loaded inside the dequant loop above).
            # One PSUM tile per (m-tile, chunk) keeps all 8 banks rotating. ---
            for mb in range(NB):
                m0 = mb * M_BLK
                if mb % 4 == 0 and mb > 0:
                    xkh_slabs = {}
                    for t in range(KT):
                        xk = xk_pool.tile([128, SLAB], f32, tag="xk")
                        nc.sync.dma_start(
                            xk[:], xt_d[t * 128 : (t + 1) * 128, m0 : m0 + SLAB]
                        )
                        xkh = xkh_pool.tile([128, SLAB], f16, tag="xkh")
                        nc.scalar.copy(xkh[:], xk[:])
                        xkh_slabs[t] = xkh
                half = (mb % 4) * M_BLK
                pos = [
                    pout_pool.tile([128, w], f32, tag="po", name=f"po_{mb}_{ci}")
                    for ci, (o, w) in enumerate(chunks)
                ]
                for t in range(KT):
                    xkh = xkh_slabs[t]
                    for ci, (o, w) in enumerate(chunks):
                        nc.tensor.matmul(
                            pos[ci][:],
                            xkh[:, half : half + 128],
                            w_tiles[t][:, o : o + w],
                            start=(t == 0),
                            stop=(t == KT - 1),
                        )
                # per-chunk drain + DMA (banks and staging free ASAP);
                # engines alternate per m-tile (gpsimd has no PSUM port)
                for ci, (o, w) in enumerate(chunks):
                    outt = out_pool.tile([128, w], f32, tag="outt", name=f"ot_{mb}_{ci}")
                    if mb == NB - 1:
                        # tail: split each drain across both engines so the
                        # final DMA starts as early as possible
                        h = w // 2
                        nc.scalar.copy(outt[:, 0:h], pos[ci][:, 0:h])
                        nc.vector.tensor_copy(outt[:, h:w], pos[ci][:, h:w])
                    elif mb % 2 == 0:
                        nc.scalar.copy(outt[:], pos[ci][:])
                    else:
                        nc.vector.tensor_copy(outt[:], pos[ci][:])
                    nc.sync.dma_start(
                        out_d[m0 : m0 + M_BLK, o : o + w], outt[:]
                    )

    nc.compile()
    return nc


_CACHE = {}


def _get_nc():
    if "nc" not in _CACHE:
        _CACHE["nc"] = build_nc()
    return _CACHE["nc"]


def shard_inputs(x, qweight, qzeros, scales):
    x = np.asarray(x, dtype=np.float32).reshape(M_ROWS, IN_F)
    xt = np.ascontiguousarray(x.T)
    qweight = np.asarray(qweight)
    qzeros = np.asarray(qzeros)
    scales = np.asarray(scales)
    pz = OUT_SHARD // 8
    k = np.arange(IN_F)
    in_maps = []
    for c in range(N_CORES):
        lo, hi = c * OUT_SHARD, (c + 1) * OUT_SHARD
        # row k of the device qweight tensor is the u16 half (low/high) of
        # packed int32 row k//8 that holds weight k's nibble — a pure gather
        qw16 = (
            np.ascontiguousarray(qweight[:, lo:hi])
            .view(np.uint16)
            .reshape(IN_F // 8, OUT_SHARD, 2)
        )
        qhost = np.ascontiguousarray(qw16[k // 8, :, (k % 8) // 4])
        in_maps.append(
            {
                "xt": xt,
                "qweight": qhost,
                "qzeros": np.ascontiguousarray(qzeros[:, c * pz : (c + 1) * pz]),
                "scales": np.ascontiguousarray(scales[:, lo:hi]),
            }
        )
    return in_maps


def gather_outputs(results):
    out = np.empty((M_ROWS, OUT_F), np.float32)
    for c in range(N_CORES):
        out[:, c * OUT_SHARD : (c + 1) * OUT_SHARD] = results[c]["out"]
    return out.reshape(B, S, OUT_F)


def kernel(x, qweight, qzeros, scales):
    in_maps = shard_inputs(x, qweight, qzeros, scales)
    res = run_bass_kernel_spmd(_get_nc(), in_maps, core_ids=list(range(N_CORES)))
    return gather_outputs(res.results)
